# revision 3
# baseline (speedup 1.0000x reference)
"""Trainium2 Bass kernel (raw Bass): per-class precision/recall sums.

Computes, for pred/gt 0-1 indicator tensors of shape [N, C]:
    intersection = sum_n pred*gt   [C]
    pred_sum     = sum_n pred      [C]
    gt_sum       = sum_n gt        [C]
    precisions   = (intersection + EPS) / (pred_sum + EPS)
    recalls      = (intersection + EPS) / (gt_sum + EPS)

Sharding: rows split across 8 NeuronCores. The host re-encodes each
core's chunk as fp8(e5m2) -- exact for 0/1 -- in 228-column blocks
    [pred(7 rows x 16 cls) | 1.0 | 0 | gt(same 7 rows) | 1.0 | 0]
staged as x[128, 592, 228] (rows per partition padded 4096 -> 592*7
with zeros; zero rows only pollute the ignored ones*ones cell). The
228 (= 0 mod 4) block width keeps every weight window 4-byte aligned
so Fast-Weight-Load stays on.

Device: one accumulating matmul per block does ALL the math:
    W = block cols 0:128   = [pred 112 | one | pad | 14 junk cols]
    R = block cols 114:227 = [gt 112 | one]
    psum[j, n] += sum_k W[k, j] * R[k, n]
  diag j=n<112   -> intersection per (r, c) slot
  col 112, j<112 -> pred sums per slot
  row 112, n<112 -> gt sums per slot
  rows 113-127, cell (112,112): junk, ignored on host.

The whole ~132 KiB/partition payload fits in SBUF, so input DMAs are
issued up front with no recycling, split across THREE DMA queues
(gpsimd SWDGE + sync/scalar HWDGE) so each SDMA engine keeps several
descriptors in flight (a single queue measured only 245 GB/s of the
358 GB/s per-core HBM limit). Small first chunks start the PE early;
small last chunks shorten the tail. PE chases per-chunk semaphores.
Epilogue: DVE copies psum -> SBUF, sync-DMA out [128, 113] fp32
partials; the host folds the 7 row-groups and sums cores in float64
(exact integer arithmetic end to end).
"""

from contextlib import ExitStack

import numpy as np

N_CORES = 8
N_ROWS, C = 4194304, 16
ROWS_PER_CORE = N_ROWS // N_CORES   # 524288
EPS = np.float32(1e-6)

P = 128
RPP = ROWS_PER_CORE // P            # 4096 rows per partition
R_GRP = 7                           # row-groups per block
D = R_GRP * C                       # 112 data cols per tensor per block
BLK_W = 228                         # 4-byte aligned block width
GT_OFF = D + 2                      # 114
M_OUT = D + 1                       # 113 meaningful out rows/cols
W_COLS = 128                        # weight window (FWL needs 128)
N_BLOCKS = 592                      # 592*7 = 4144 row slots (48 pad)

# DMA chunk sizes (blocks); chunk i goes to queue i % 3.
CHUNKS = [8, 8, 8] + [32] * 17 + [8, 8, 8]
assert sum(CHUNKS) == N_BLOCKS
N_QUEUES = 3

ONE_E5M2 = np.uint8(0x3C)           # bit pattern of 1.0 in fp8 e5m2

_CACHE = {}
LAST_RUN = None  # BassKernelResults of the most recent run (for test harness)


def _build_nc():
    import concourse.bass as bass
    import concourse.mybir as mybir

    f32 = mybir.dt.float32
    f8 = mybir.dt.float8e5

    nc = bass.Bass()
    x_d = nc.dram_tensor("x", [P, N_BLOCKS, BLK_W], f8, kind="ExternalInput")
    out_d = nc.dram_tensor("out", [P, M_OUT], f32, kind="ExternalOutput")

    starts = np.concatenate([[0], np.cumsum(CHUNKS)])
    chunk_of_queue = [
        [i for i in range(len(CHUNKS)) if i % N_QUEUES == q]
        for q in range(N_QUEUES)
    ]

    ctx = ExitStack()
    with ctx:
        data = ctx.enter_context(nc.sbuf_tensor("data", [P, N_BLOCKS, BLK_W], f8))
        res = ctx.enter_context(nc.sbuf_tensor("res", [P, M_OUT], f32))
        ps = ctx.enter_context(nc.psum_tensor([P, M_OUT], f32))

        csems = [
            ctx.enter_context(nc.semaphore(name=f"c{i}"))
            for i in range(len(CHUNKS))
        ]
        pe_sem = ctx.enter_context(nc.semaphore(name="pe"))
        dve_sem = ctx.enter_context(nc.semaphore(name="dve"))
        out_sem = ctx.enter_context(nc.semaphore(name="outd"))
        block = ctx.enter_context(nc.Block())

        def issue_queue(eng, q):
            for i in chunk_of_queue[q]:
                s, e = int(starts[i]), int(starts[i + 1])
                eng.dma_start(
                    data[:, s:e, :], x_d[:, s:e, :]
                ).then_inc(csems[i], 16)

        @block.gpsimd
        def _(gpsimd):
            issue_queue(gpsimd, 0)

        @block.sync
        def _(sync):
            issue_queue(sync, 1)
            # output DMA after DVE finishes the psum -> SBUF copy
            sync.wait_ge(dve_sem, 1)
            sync.dma_start(out_d[:, :], res[:, :]).then_inc(out_sem, 16)
            sync.wait_ge(out_sem, 16)

        @block.scalar
        def _(scalar):
            issue_queue(scalar, 2)

        @block.tensor
        def _(tensor):
            inst = None
            for i in range(len(CHUNKS)):
                tensor.wait_ge(csems[i], 16)
                for b in range(int(starts[i]), int(starts[i + 1])):
                    inst = nc.tensor.matmul(
                        ps[:, :],
                        data[:, b, 0:W_COLS],
                        data[:, b, GT_OFF:GT_OFF + M_OUT],
                        start=(b == 0),
                        stop=(b == N_BLOCKS - 1),
                    )
            inst.then_inc(pe_sem, 1)

        @block.vector
        def _(vector):
            vector.wait_ge(pe_sem, 1)
            vector.tensor_copy(res[:, :], ps[:, :])
            vector.nop().then_inc(dve_sem, 1)

    return nc


def _get_nc():
    if "nc" not in _CACHE:
        _CACHE["nc"] = _build_nc()
    return _CACHE["nc"]


def _stage_core(pred_u8, gt_u8):
    """pred_u8/gt_u8: [ROWS_PER_CORE, C] uint8 0/1 -> x[P, N_BLOCKS, BLK_W]
    fp8e5m2 bit pattern (as uint8)."""
    x = np.zeros((P, N_BLOCKS, BLK_W), dtype=np.uint8)
    pad = np.zeros((P, N_BLOCKS * R_GRP - RPP, C), dtype=np.uint8)

    pb = np.concatenate([pred_u8.reshape(P, RPP, C), pad], axis=1)
    x[:, :, 0:D] = pb.reshape(P, N_BLOCKS, D) * ONE_E5M2
    x[:, :, D] = ONE_E5M2

    gb = np.concatenate([gt_u8.reshape(P, RPP, C), pad], axis=1)
    x[:, :, GT_OFF:GT_OFF + D] = gb.reshape(P, N_BLOCKS, D) * ONE_E5M2
    x[:, :, GT_OFF + D] = ONE_E5M2
    return x


def kernel(pred, gt, **run_kwargs):
    global LAST_RUN
    import ml_dtypes
    from concourse.bass_utils import run_bass_kernel_spmd

    pred = np.asarray(pred)
    gt = np.asarray(gt)
    assert pred.shape == (N_ROWS, C) and gt.shape == (N_ROWS, C)

    pred_u8 = pred.astype(np.uint8)   # 0/1
    gt_u8 = gt.astype(np.uint8)

    in_maps = []
    for i in range(N_CORES):
        sl = slice(i * ROWS_PER_CORE, (i + 1) * ROWS_PER_CORE)
        x = _stage_core(pred_u8[sl], gt_u8[sl])
        in_maps.append({"x": x.view(ml_dtypes.float8_e5m2)})

    nc = _get_nc()
    br = run_bass_kernel_spmd(nc, in_maps, core_ids=list(range(N_CORES)),
                              **run_kwargs)
    LAST_RUN = br

    # Sum the [128, 113] per-core partials exactly, then fold the
    # 7 row-groups per class.
    T = np.zeros((P, M_OUT), dtype=np.float64)
    for r in br.results:
        T += np.asarray(r["out"], dtype=np.float64)

    diag = np.diagonal(T)[:D]                       # intersection slots
    intersection = diag.reshape(R_GRP, C).sum(axis=0).astype(np.float32)
    pred_sum = T[:D, D].reshape(R_GRP, C).sum(axis=0).astype(np.float32)
    gt_sum = T[D, :D].reshape(R_GRP, C).sum(axis=0).astype(np.float32)

    recalls = (intersection + EPS) / (gt_sum + EPS)
    precisions = (intersection + EPS) / (pred_sum + EPS)
    return (precisions, recalls, intersection, gt_sum, pred_sum)


# revision 6
# speedup vs baseline: 1.1936x; 1.1936x over previous
"""Trainium2 Bass kernel (raw Bass): per-class precision/recall sums.

Computes, for pred/gt 0-1 indicator tensors of shape [N, C]:
    intersection = sum_n pred*gt   [C]
    pred_sum     = sum_n pred      [C]
    gt_sum       = sum_n gt        [C]
    precisions   = (intersection + EPS) / (pred_sum + EPS)
    recalls      = (intersection + EPS) / (gt_sum + EPS)

Sharding: rows split across 8 NeuronCores. The host re-encodes each
core's chunk as fp8(e5m2) -- exact for 0/1 -- in 228-column blocks
    [pred(7 rows x 16 cls) | 1.0 | 0 | gt(same 7 rows) | 1.0 | 0]
staged as x[128, 592, 228] (rows per partition padded 4096 -> 592*7
with zeros; zero rows only pollute the ignored ones*ones cell). The
228 (= 0 mod 4) block width keeps every weight window 4-byte aligned
so Fast-Weight-Load stays on.

Device: one accumulating matmul per block does ALL the math:
    W = block cols 0:128   = [pred 112 | one | pad | 14 junk cols]
    R = block cols 114:227 = [gt 112 | one]
    psum[j, n] += sum_k W[k, j] * R[k, n]
  diag j=n<112   -> intersection per (r, c) slot
  col 112, j<112 -> pred sums per slot
  row 112, n<112 -> gt sums per slot
  rows 113-127, cell (112,112): junk, ignored on host.

The whole ~132 KiB/partition payload fits in SBUF, so input DMAs are
issued up front with no recycling, split across THREE DMA queues
(gpsimd SWDGE + sync/scalar HWDGE) so each SDMA engine keeps several
descriptors in flight (a single queue measured only 245 GB/s of the
358 GB/s per-core HBM limit). Small first chunks start the PE early;
small last chunks shorten the tail. PE chases per-chunk semaphores.
Epilogue: DVE copies psum -> SBUF, sync-DMA out [128, 113] fp32
partials; the host folds the 7 row-groups and sums cores in float64
(exact integer arithmetic end to end).
"""

from contextlib import ExitStack

import numpy as np

N_CORES = 8
N_ROWS, C = 4194304, 16
ROWS_PER_CORE = N_ROWS // N_CORES   # 524288
EPS = np.float32(1e-6)

P = 128
RPP = ROWS_PER_CORE // P            # 4096 rows per partition
R_GRP = 7                           # row-groups per block
D = R_GRP * C                       # 112 data cols per tensor per block
BLK_W = 228                         # 4-byte aligned block width
GT_OFF = D + 2                      # 114
M_OUT = D + 1                       # 113 meaningful out rows/cols
W_COLS = 128                        # weight window (FWL needs 128)
N_BLOCKS = 592                      # 592*7 = 4144 row slots (48 pad)

# DMA chunk sizes (blocks), all on the sync HWDGE queue: small head so
# the PE starts early, big middle for descriptor efficiency (fewer,
# larger descriptors), small tail so the PE finishes right behind the
# last byte.
CHUNKS = [8, 24, 80, 96, 96, 96, 96, 72, 16, 8]
assert sum(CHUNKS) == N_BLOCKS

ONE_E5M2 = np.uint8(0x3C)           # bit pattern of 1.0 in fp8 e5m2

_CACHE = {}
LAST_RUN = None  # BassKernelResults of the most recent run (for test harness)


def _build_nc():
    import concourse.bass as bass
    import concourse.mybir as mybir

    f32 = mybir.dt.float32
    f8 = mybir.dt.float8e5

    nc = bass.Bass()
    x_d = nc.dram_tensor("x", [P, N_BLOCKS, BLK_W], f8, kind="ExternalInput")
    out_d = nc.dram_tensor("out", [P, M_OUT], f32, kind="ExternalOutput")

    starts = np.concatenate([[0], np.cumsum(CHUNKS)])

    ctx = ExitStack()
    with ctx:
        data = ctx.enter_context(nc.sbuf_tensor("data", [P, N_BLOCKS, BLK_W], f8))
        res = ctx.enter_context(nc.sbuf_tensor("res", [P, M_OUT], f32))
        ps = ctx.enter_context(nc.psum_tensor([P, M_OUT], f32))

        csems = [
            ctx.enter_context(nc.semaphore(name=f"c{i}"))
            for i in range(len(CHUNKS))
        ]
        pe_sem = ctx.enter_context(nc.semaphore(name="pe"))
        dve_sem = ctx.enter_context(nc.semaphore(name="dve"))
        out_sem = ctx.enter_context(nc.semaphore(name="outd"))
        block = ctx.enter_context(nc.Block())

        @block.sync
        def _(sync):
            # all input DMAs on the sync HWDGE queue (no SWDGE
            # descriptor-ring wrap stalls, faster initiation)
            for i in range(len(CHUNKS)):
                s, e = int(starts[i]), int(starts[i + 1])
                sync.dma_start(
                    data[:, s:e, :], x_d[:, s:e, :]
                ).then_inc(csems[i], 16)

        @block.scalar
        def _(scalar):
            # output DMA on the scalar HWDGE queue after DVE finishes
            # the psum -> SBUF copy
            scalar.wait_ge(dve_sem, 1)
            scalar.dma_start(out_d[:, :], res[:, :]).then_inc(out_sem, 16)
            scalar.wait_ge(out_sem, 16)

        @block.tensor
        def _(tensor):
            inst = None
            for i in range(len(CHUNKS)):
                tensor.wait_ge(csems[i], 16)
                for b in range(int(starts[i]), int(starts[i + 1])):
                    inst = nc.tensor.matmul(
                        ps[:, :],
                        data[:, b, 0:W_COLS],
                        data[:, b, GT_OFF:GT_OFF + M_OUT],
                        start=(b == 0),
                        stop=(b == N_BLOCKS - 1),
                    )
            inst.then_inc(pe_sem, 1)

        @block.vector
        def _(vector):
            vector.wait_ge(pe_sem, 1)
            vector.tensor_copy(res[:, :], ps[:, :])
            vector.nop().then_inc(dve_sem, 1)

    return nc


def _get_nc():
    if "nc" not in _CACHE:
        _CACHE["nc"] = _build_nc()
    return _CACHE["nc"]


def _stage_core(pred_u8, gt_u8):
    """pred_u8/gt_u8: [ROWS_PER_CORE, C] uint8 0/1 -> x[P, N_BLOCKS, BLK_W]
    fp8e5m2 bit pattern (as uint8)."""
    x = np.zeros((P, N_BLOCKS, BLK_W), dtype=np.uint8)
    pad = np.zeros((P, N_BLOCKS * R_GRP - RPP, C), dtype=np.uint8)

    pb = np.concatenate([pred_u8.reshape(P, RPP, C), pad], axis=1)
    x[:, :, 0:D] = pb.reshape(P, N_BLOCKS, D) * ONE_E5M2
    x[:, :, D] = ONE_E5M2

    gb = np.concatenate([gt_u8.reshape(P, RPP, C), pad], axis=1)
    x[:, :, GT_OFF:GT_OFF + D] = gb.reshape(P, N_BLOCKS, D) * ONE_E5M2
    x[:, :, GT_OFF + D] = ONE_E5M2
    return x


def kernel(pred, gt, **run_kwargs):
    global LAST_RUN
    import ml_dtypes
    from concourse.bass_utils import run_bass_kernel_spmd

    pred = np.asarray(pred)
    gt = np.asarray(gt)
    assert pred.shape == (N_ROWS, C) and gt.shape == (N_ROWS, C)

    pred_u8 = pred.astype(np.uint8)   # 0/1
    gt_u8 = gt.astype(np.uint8)

    in_maps = []
    for i in range(N_CORES):
        sl = slice(i * ROWS_PER_CORE, (i + 1) * ROWS_PER_CORE)
        x = _stage_core(pred_u8[sl], gt_u8[sl])
        in_maps.append({"x": x.view(ml_dtypes.float8_e5m2)})

    nc = _get_nc()
    br = run_bass_kernel_spmd(nc, in_maps, core_ids=list(range(N_CORES)),
                              **run_kwargs)
    LAST_RUN = br

    # Sum the [128, 113] per-core partials exactly, then fold the
    # 7 row-groups per class.
    T = np.zeros((P, M_OUT), dtype=np.float64)
    for r in br.results:
        T += np.asarray(r["out"], dtype=np.float64)

    diag = np.diagonal(T)[:D]                       # intersection slots
    intersection = diag.reshape(R_GRP, C).sum(axis=0).astype(np.float32)
    pred_sum = T[:D, D].reshape(R_GRP, C).sum(axis=0).astype(np.float32)
    gt_sum = T[D, :D].reshape(R_GRP, C).sum(axis=0).astype(np.float32)

    recalls = (intersection + EPS) / (gt_sum + EPS)
    precisions = (intersection + EPS) / (pred_sum + EPS)
    return (precisions, recalls, intersection, gt_sum, pred_sum)


# revision 8
# speedup vs baseline: 1.2755x; 1.0686x over previous
"""Trainium2 Bass kernel (raw Bass): per-class precision/recall sums.

Computes, for pred/gt 0-1 indicator tensors of shape [N, C]:
    intersection = sum_n pred*gt   [C]
    pred_sum     = sum_n pred      [C]
    gt_sum       = sum_n gt        [C]
    precisions   = (intersection + EPS) / (pred_sum + EPS)
    recalls      = (intersection + EPS) / (gt_sum + EPS)

Sharding: rows split across 8 NeuronCores. The host re-encodes each
core's chunk as fp8(e5m2) -- exact for 0/1 -- in 228-column blocks
    [pred(7 rows x 16 cls) | 1.0 | 0 | gt(same 7 rows) | 1.0 | 0]
staged as x[128, 592, 228] (rows per partition padded 4096 -> 592*7
with zeros; zero rows only pollute the ignored ones*ones cell). The
228 (= 0 mod 4) block width keeps every weight window 4-byte aligned
so Fast-Weight-Load stays on.

Device: one accumulating matmul per block does ALL the math:
    W = block cols 0:128   = [pred 112 | one | pad | 14 junk cols]
    R = block cols 114:227 = [gt 112 | one]
    psum[j, n] += sum_k W[k, j] * R[k, n]
  diag j=n<112   -> intersection per (r, c) slot
  col 112, j<112 -> pred sums per slot
  row 112, n<112 -> gt sums per slot
  rows 113-127, cell (112,112): junk, ignored on host.

The whole ~132 KiB/partition payload fits in SBUF, so input DMAs are
issued up front with no recycling, split across THREE DMA queues
(gpsimd SWDGE + sync/scalar HWDGE) so each SDMA engine keeps several
descriptors in flight (a single queue measured only 245 GB/s of the
358 GB/s per-core HBM limit). Small first chunks start the PE early;
small last chunks shorten the tail. PE chases per-chunk semaphores.
Epilogue: DVE copies psum -> SBUF, sync-DMA out [128, 113] fp32
partials; the host folds the 7 row-groups and sums cores in float64
(exact integer arithmetic end to end).
"""

from contextlib import ExitStack

import numpy as np

N_CORES = 8
N_ROWS, C = 4194304, 16
ROWS_PER_CORE = N_ROWS // N_CORES   # 524288
EPS = np.float32(1e-6)

P = 128
RPP = ROWS_PER_CORE // P            # 4096 rows per partition
R_GRP = 7                           # row-groups per block
D = R_GRP * C                       # 112 data cols per tensor per block
BLK_W = 228                         # 4-byte aligned block width
GT_OFF = D + 2                      # 114
M_OUT = D + 1                       # 113 meaningful out rows/cols
W_COLS = 128                        # weight window (FWL needs 128)
N_BLOCKS = 592                      # 592*7 = 4144 row slots (48 pad)

# DMA chunk sizes (blocks): small head so the PE starts early, big
# middle for descriptor efficiency, finer tail so the PE finishes right
# behind the last byte. Chunk 0 goes on the sync HWDGE queue (starts
# ~1.3 us before the SWDGE path); the rest go on gpsimd SWDGE, which
# keeps all 16 SDMA engines uniform (HWDGE makes engine 15 ~17% slower
# -- descriptor fetches contend with its AXI port -- and every chunk
# semaphore waits on the slowest engine).
CHUNKS = [8, 24, 80, 96, 96, 96, 80, 48, 24, 16, 8, 8, 8]
assert sum(CHUNKS) == N_BLOCKS

ONE_E5M2 = np.uint8(0x3C)           # bit pattern of 1.0 in fp8 e5m2

_CACHE = {}
LAST_RUN = None  # BassKernelResults of the most recent run (for test harness)


def _build_nc():
    import concourse.bass as bass
    import concourse.mybir as mybir

    f32 = mybir.dt.float32
    f8 = mybir.dt.float8e5

    nc = bass.Bass()
    x_d = nc.dram_tensor("x", [P, N_BLOCKS, BLK_W], f8, kind="ExternalInput")
    out_d = nc.dram_tensor("out", [P, M_OUT], f32, kind="ExternalOutput")

    starts = np.concatenate([[0], np.cumsum(CHUNKS)])

    ctx = ExitStack()
    with ctx:
        data = ctx.enter_context(nc.sbuf_tensor("data", [P, N_BLOCKS, BLK_W], f8))
        res = ctx.enter_context(nc.sbuf_tensor("res", [P, M_OUT], f32))
        ps = ctx.enter_context(nc.psum_tensor([P, M_OUT], f32))

        csems = [
            ctx.enter_context(nc.semaphore(name=f"c{i}"))
            for i in range(len(CHUNKS))
        ]
        pe_sem = ctx.enter_context(nc.semaphore(name="pe"))
        dve_sem = ctx.enter_context(nc.semaphore(name="dve"))
        out_sem = ctx.enter_context(nc.semaphore(name="outd"))
        block = ctx.enter_context(nc.Block())

        @block.sync
        def _(sync):
            s, e = int(starts[0]), int(starts[1])
            sync.dma_start(
                data[:, s:e, :], x_d[:, s:e, :]
            ).then_inc(csems[0], 16)

        @block.gpsimd
        def _(gpsimd):
            for i in range(1, len(CHUNKS)):
                s, e = int(starts[i]), int(starts[i + 1])
                gpsimd.dma_start(
                    data[:, s:e, :], x_d[:, s:e, :]
                ).then_inc(csems[i], 16)

        @block.scalar
        def _(scalar):
            # output DMA on the scalar HWDGE queue after DVE finishes
            # the psum -> SBUF copy
            scalar.wait_ge(dve_sem, 1)
            scalar.dma_start(out_d[:, :], res[:, :]).then_inc(out_sem, 16)
            scalar.wait_ge(out_sem, 16)

        @block.tensor
        def _(tensor):
            inst = None
            for i in range(len(CHUNKS)):
                tensor.wait_ge(csems[i], 16)
                for b in range(int(starts[i]), int(starts[i + 1])):
                    inst = nc.tensor.matmul(
                        ps[:, :],
                        data[:, b, 0:W_COLS],
                        data[:, b, GT_OFF:GT_OFF + M_OUT],
                        start=(b == 0),
                        stop=(b == N_BLOCKS - 1),
                    )
            inst.then_inc(pe_sem, 1)

        @block.vector
        def _(vector):
            vector.wait_ge(pe_sem, 1)
            vector.tensor_copy(res[:, :], ps[:, :])
            vector.nop().then_inc(dve_sem, 1)

    return nc


def _get_nc():
    if "nc" not in _CACHE:
        _CACHE["nc"] = _build_nc()
    return _CACHE["nc"]


def _stage_core(pred_u8, gt_u8):
    """pred_u8/gt_u8: [ROWS_PER_CORE, C] uint8 0/1 -> x[P, N_BLOCKS, BLK_W]
    fp8e5m2 bit pattern (as uint8)."""
    x = np.zeros((P, N_BLOCKS, BLK_W), dtype=np.uint8)
    pad = np.zeros((P, N_BLOCKS * R_GRP - RPP, C), dtype=np.uint8)

    pb = np.concatenate([pred_u8.reshape(P, RPP, C), pad], axis=1)
    x[:, :, 0:D] = pb.reshape(P, N_BLOCKS, D) * ONE_E5M2
    x[:, :, D] = ONE_E5M2

    gb = np.concatenate([gt_u8.reshape(P, RPP, C), pad], axis=1)
    x[:, :, GT_OFF:GT_OFF + D] = gb.reshape(P, N_BLOCKS, D) * ONE_E5M2
    x[:, :, GT_OFF + D] = ONE_E5M2
    return x


def kernel(pred, gt, **run_kwargs):
    global LAST_RUN
    import ml_dtypes
    from concourse.bass_utils import run_bass_kernel_spmd

    pred = np.asarray(pred)
    gt = np.asarray(gt)
    assert pred.shape == (N_ROWS, C) and gt.shape == (N_ROWS, C)

    pred_u8 = pred.astype(np.uint8)   # 0/1
    gt_u8 = gt.astype(np.uint8)

    in_maps = []
    for i in range(N_CORES):
        sl = slice(i * ROWS_PER_CORE, (i + 1) * ROWS_PER_CORE)
        x = _stage_core(pred_u8[sl], gt_u8[sl])
        in_maps.append({"x": x.view(ml_dtypes.float8_e5m2)})

    nc = _get_nc()
    br = run_bass_kernel_spmd(nc, in_maps, core_ids=list(range(N_CORES)),
                              **run_kwargs)
    LAST_RUN = br

    # Sum the [128, 113] per-core partials exactly, then fold the
    # 7 row-groups per class.
    T = np.zeros((P, M_OUT), dtype=np.float64)
    for r in br.results:
        T += np.asarray(r["out"], dtype=np.float64)

    diag = np.diagonal(T)[:D]                       # intersection slots
    intersection = diag.reshape(R_GRP, C).sum(axis=0).astype(np.float32)
    pred_sum = T[:D, D].reshape(R_GRP, C).sum(axis=0).astype(np.float32)
    gt_sum = T[D, :D].reshape(R_GRP, C).sum(axis=0).astype(np.float32)

    recalls = (intersection + EPS) / (gt_sum + EPS)
    precisions = (intersection + EPS) / (pred_sum + EPS)
    return (precisions, recalls, intersection, gt_sum, pred_sum)
